# revision 1
# baseline (speedup 1.0000x reference)
"""Multi-head attention (B=2, S=2048, D=1024, H=16, d_k=64) on 8 NeuronCores.

Sharding: 8 cores = 2 batches x 4 head-groups (4 heads each).
Core c handles batch b = c//4 and heads 4*(c%4) .. 4*(c%4)+4 (feature
slice of width F=256). Each core computes its partial output-projection
contribution [S, D]; the host sums the 4 head-group partials per batch
and adds b4 (the "all-reduce" of the row-sharded W4 projection).

Device dataflow works in a "transposed world" so every matmul operand
is in its natural PE layout (contraction on partitions), with zero
on-device transposes:
  qT = W1g @ x_q.T  [F, S]   (lhsT = W1g.T host-prepped, rhs = x_q.T host-prepped)
  kT = W2g @ x_k.T  [F, S]
  v  = x_v @ W3g.T  [S, F]   (lhsT = x_v.T, rhs = W3g.T; bias via K=1 ones matmul)
  scoresT_h = kT_h.T @ qT_h        [S_keys, S_q]   (K = d_k = 64; 2 heads packed
                                                    in PE row groups 0:64 / 64:128)
  attnT = exp(scoresT / 8)          ACT, PSUM->SBUF bf16, no max subtraction
                                    (scores ~ N(0,1); max|score/8| ~ 10 -> safe in f32)
  pv = [v_h | ones].T @ attnT      [65, S_q]; row 64 = softmax denominator
  outT_h = pv[0:64] * (1/pv[64])   per-query normalization (flash-style, post-PV)
  partial = outT.T @ W4g.T         [S, D]  (lhsT = outT, rhs = W4g.T host-prepped)

All matmuls bf16 with f32 PSUM accumulation (validated 0.6% L2 rel err).

Schedule: attention is processed per (head-pair, query-half) window; within
a window, scores MMs (kt) and PV MMs (kt-1) interleave per key tile so the
PE stream has no multi-us stalls (keeps the HAM clock gate warm) while ACT
exp paces the pipeline. PSUM: scores 2x2 banks + PV accumulators 2x2 banks;
projection / output-projection psum recycles the same tags.
"""

import numpy as np
import ml_dtypes

import concourse.bass as bass
import concourse.mybir as mybir
import concourse.tile as tile
from concourse import bacc
from concourse.bass_utils import run_bass_kernel_spmd

BF16 = ml_dtypes.bfloat16
F32 = mybir.dt.float32
BF = mybir.dt.bfloat16

B, S, D = 2, 2048, 1024
H_CORE = 4          # heads per core
DK = 64             # head dim
F = H_CORE * DK     # features per core = 256
P = 128             # partitions
KB = D // P         # k blocks in D contraction = 8
SM = S // P         # seq tiles of 128 = 16
QW = 1024           # query window width
NQW = S // QW       # query windows = 2
N_CORES = 8


DEBUG_TAPS = False
EXACT_RECIP = True


def _build_kernel():
    nc = bacc.Bacc(
        "TRN2",
        target_bir_lowering=False,
        debug=False,
        enable_asserts=False,
        num_devices=N_CORES,
    )

    xq = nc.dram_tensor("xq_t", [D, S], BF, kind="ExternalInput").ap()
    xk = nc.dram_tensor("xk_t", [D, S], BF, kind="ExternalInput").ap()
    xv = nc.dram_tensor("xv_t", [D, S], BF, kind="ExternalInput").ap()
    w1 = nc.dram_tensor("w1t", [D, F], BF, kind="ExternalInput").ap()
    w2 = nc.dram_tensor("w2t", [D, F], BF, kind="ExternalInput").ap()
    w3 = nc.dram_tensor("w3t", [D, F], BF, kind="ExternalInput").ap()
    w4 = nc.dram_tensor("w4t", [F, D], BF, kind="ExternalInput").ap()
    b1 = nc.dram_tensor("b1c", [P, F // P], F32, kind="ExternalInput").ap()
    b2 = nc.dram_tensor("b2c", [P, F // P], F32, kind="ExternalInput").ap()
    b3 = nc.dram_tensor("b3r", [1, F], BF, kind="ExternalInput").ap()
    out = nc.dram_tensor("out", [S, D], F32, kind="ExternalOutput").ap()
    taps = None
    if DEBUG_TAPS:
        taps = {
            "dbg_qT0": nc.dram_tensor("dbg_qT0", [P, S], BF, kind="ExternalOutput").ap(),
            "dbg_kT0": nc.dram_tensor("dbg_kT0", [P, S], BF, kind="ExternalOutput").ap(),
            "dbg_v0": nc.dram_tensor("dbg_v0", [P, 260], BF, kind="ExternalOutput").ap(),
            "dbg_outT0": nc.dram_tensor("dbg_outT0", [P, S], BF, kind="ExternalOutput").ap(),
        }

    with tile.TileContext(nc) as tc:
        _body(tc, xq, xk, xv, w1, w2, w3, w4, b1, b2, b3, out, taps)

    nc.compile()
    return nc


def _body(tc, xq, xk, xv, w1, w2, w3, w4, b1, b2, b3, out, taps=None):
    nc = tc.nc
    MF = F // P  # m tiles for the F=256 feature dim = 2

    with (
        tc.tile_pool(name="wpool", bufs=1) as wpool,
        tc.tile_pool(name="xt", bufs=20) as xt_pool,
        tc.tile_pool(name="persist", bufs=1) as persist,
        tc.tile_pool(name="attn", bufs=6) as attn_pool,
        tc.tile_pool(name="small", bufs=4) as small,
        tc.tile_pool(name="stage", bufs=2) as stage,
        tc.tile_pool(name="psum", bufs=1, space="PSUM") as psum,
    ):
        # ---- weights / constants to SBUF ----
        w1_sb = [wpool.tile([P, F], BF, name=f"w1_{k}", tag=f"w1_{k}") for k in range(KB)]
        w2_sb = [wpool.tile([P, F], BF, name=f"w2_{k}", tag=f"w2_{k}") for k in range(KB)]
        w3_sb = [wpool.tile([P, F], BF, name=f"w3_{k}", tag=f"w3_{k}") for k in range(KB)]
        w4_sb = [wpool.tile([P, D], BF, name=f"w4_{k}", tag=f"w4_{k}") for k in range(MF)]
        for k in range(KB):
            nc.sync.dma_start(w1_sb[k][:], w1[k * P:(k + 1) * P, :])
            nc.sync.dma_start(w2_sb[k][:], w2[k * P:(k + 1) * P, :])
            nc.sync.dma_start(w3_sb[k][:], w3[k * P:(k + 1) * P, :])
        for k in range(MF):
            nc.sync.dma_start(w4_sb[k][:], w4[k * P:(k + 1) * P, :])
        b1_sb = wpool.tile([P, MF], F32, name="b1_sb", tag="b1_sb")
        b2_sb = wpool.tile([P, MF], F32, name="b2_sb", tag="b2_sb")
        b3_sb = wpool.tile([1, F], BF, name="b3_sb", tag="b3_sb")
        nc.sync.dma_start(b1_sb[:], b1[:])
        nc.sync.dma_start(b2_sb[:], b2[:])
        nc.sync.dma_start(b3_sb[:], b3[:])
        ones_row = wpool.tile([1, P], BF, name="ones_row", tag="ones_row")
        nc.vector.memset(ones_row[:], 1.0)

        # persistent activations
        qT = [persist.tile([P, S], BF, name=f"qT_{m}", tag=f"qT_{m}") for m in range(MF)]
        kT = [persist.tile([P, S], BF, name=f"kT_{m}", tag=f"kT_{m}") for m in range(MF)]
        # v with interleaved ones columns: per head h, cols 65h..65h+63 = v_h,
        # col 65h+64 = 1.0 (softmax denominator trick)
        VW = H_CORE * (DK + 1)  # 260
        v_sb = [persist.tile([P, VW], BF, name=f"v_{s}", tag=f"v_{s}") for s in range(SM)]
        for s in range(SM):
            for h in range(H_CORE):
                nc.vector.memset(v_sb[s][:, h * (DK + 1) + DK: h * (DK + 1) + DK + 1], 1.0)
        outT = [persist.tile([P, S], BF, name=f"outT_{m}", tag=f"outT_{m}") for m in range(MF)]

        # ---- q / k projections: qT[m][:, qw] = sum_k W1t[k][:,m].T @ xq[k][:,qw] ----
        for name, x_dram, w_sb, b_sb, dst in (
            ("q", xq, w1_sb, b1_sb, qT),
            ("k", xk, w2_sb, b2_sb, kT),
        ):
            x_sb = []
            for k in range(KB):
                t = xt_pool.tile([P, S], BF, name=f"x{name}_{k}", tag="xt")
                nc.sync.dma_start(t[:], x_dram[k * P:(k + 1) * P, :])
                x_sb.append(t)
            for m in range(MF):
                for qw in range(NQW):
                    ps = psum.tile([P, QW], F32, name=f"pp_{name}_{m}_{qw}", tag="sc", bufs=2)
                    for k in range(KB):
                        for half in range(2):
                            nc.tensor.matmul(
                                ps[:, half * 512:(half + 1) * 512],
                                w_sb[k][:, m * P:(m + 1) * P],
                                x_sb[k][:, qw * QW + half * 512: qw * QW + (half + 1) * 512],
                                start=(k == 0),
                                stop=(k == KB - 1),
                            )
                    nc.vector.tensor_scalar_add(
                        dst[m][:, qw * QW:(qw + 1) * QW], ps[:], b_sb[:, m:m + 1]
                    )

        # ---- v projection (natural layout): v[s] = xv[:, s].T @ W3t + b3 ----
        x_sb = []
        for k in range(KB):
            t = xt_pool.tile([P, S], BF, name=f"xv_{k}", tag="xt")
            nc.sync.dma_start(t[:], xv[k * P:(k + 1) * P, :])
            x_sb.append(t)
        for s in range(SM):
            ps = psum.tile([P, F], F32, name=f"pv_{s}", tag="pv", bufs=2)
            for k in range(KB):
                nc.tensor.matmul(
                    ps[:],
                    x_sb[k][:, s * P:(s + 1) * P],
                    w3_sb[k][:],
                    start=(k == 0),
                    stop=False,
                )
            # bias: += ones.T @ b3  (K=1)
            nc.tensor.matmul(ps[:], ones_row[:], b3_sb[:], start=False, stop=True)
            for h in range(H_CORE):
                nc.vector.tensor_copy(
                    v_sb[s][:, h * (DK + 1): h * (DK + 1) + DK],
                    ps[:, h * DK:(h + 1) * DK],
                )

        # ---- output projection step generator (used as PE filler + tail) ----
        def gen_w4(qts):
            for qt in qts:
                ps = psum.tile([P, D], F32, name=f"po_{qt}", tag="sc", bufs=2)
                for oc in range(D // 512):
                    for m in range(MF):
                        nc.tensor.matmul(
                            ps[:, oc * 512:(oc + 1) * 512],
                            outT[m][:, qt * P:(qt + 1) * P],
                            w4_sb[m][:, oc * 512:(oc + 1) * 512],
                            start=(m == 0),
                            stop=(m == MF - 1),
                        )
                    yield
                ob = stage.tile([P, D], F32, name=f"ob_{qt}", tag="ob")
                nc.vector.tensor_copy(ob[:], ps[:])
                nc.sync.dma_start(out[qt * P:(qt + 1) * P, :], ob[:])
                yield

        # ---- attention, per head-pair hp (heads 2hp, 2hp+1 live in qT/kT tile hp),
        #      per query window qw; scores(kt) and PV(kt-1) interleaved per key tile.
        #      filler: (start_slot, steps_per_slot, generator) for PE idle slots ----
        def window(hp, qw, filler=None):
            qsl = slice(qw * QW, (qw + 1) * QW)
            attn_t = [[None] * SM for _ in range(2)]
            pv_ps = [
                psum.tile([P, QW], F32, name=f"pvps_{hp}_{qw}_{h2}", tag="pv", bufs=2)
                for h2 in range(2)
            ]

            def emit_scores(kt):
                for h2 in range(2):
                    rsl = slice(h2 * DK, (h2 + 1) * DK)
                    ps = psum.tile([P, QW], F32, name=f"sc_{hp}_{qw}_{kt}_{h2}",
                                   tag="sc", bufs=2)
                    for half in range(2):
                        nc.tensor.matmul(
                            ps[:, half * 512:(half + 1) * 512],
                            kT[hp][rsl, kt * P:(kt + 1) * P],
                            qT[hp][rsl, qw * QW + half * 512: qw * QW + (half + 1) * 512],
                            start=True,
                            stop=True,
                        )
                    at = attn_pool.tile([P, QW], BF, name=f"at_{hp}_{qw}_{kt}_{h2}",
                                        tag="attnT", bufs=6)
                    nc.scalar.activation(
                        at[:], ps[:], mybir.ActivationFunctionType.Exp,
                        scale=1.0 / np.sqrt(DK),
                    )
                    attn_t[h2][kt] = at

            def emit_pv(kt):
                for h2 in range(2):
                    h = hp * 2 + h2
                    vsl = slice(h * (DK + 1), h * (DK + 1) + DK + 1)
                    for half in range(2):
                        nc.tensor.matmul(
                            pv_ps[h2][0:DK + 1, half * 512:(half + 1) * 512],
                            v_sb[kt][:, vsl],
                            attn_t[h2][kt][:, half * 512:(half + 1) * 512],
                            start=(kt == 0),
                            stop=(kt == SM - 1),
                        )

            emit_scores(0)
            for kt in range(1, SM):
                emit_scores(kt)
                emit_pv(kt - 1)
                if filler is not None and kt >= filler[0]:
                    for _ in range(filler[1]):
                        next(filler[2], None)
            emit_pv(SM - 1)

            # fast PSUM release: copy denominators + raw (unnormalized) outT,
            # then normalize off the critical path.
            dens, raws = [], []
            for h2 in range(2):
                den = small.tile([1, QW], F32, name=f"den_{hp}_{qw}_{h2}", tag="den", bufs=3)
                nc.vector.tensor_copy(den[:], pv_ps[h2][DK:DK + 1, :])
                dens.append(den)
            for h2 in range(2):
                raw = small.tile([DK, QW], BF, name=f"raw_{hp}_{qw}_{h2}", tag="raw", bufs=3)
                nc.vector.tensor_copy(raw[:], pv_ps[h2][0:DK, :])
                raws.append(raw)
            for h2 in range(2):
                rec = small.tile([1, QW], F32, name=f"rec_{hp}_{qw}_{h2}", tag="rec", bufs=3)
                nc.vector.reciprocal(rec[:], dens[h2][:])
                bc = small.tile([DK, QW], F32, name=f"bc_{hp}_{qw}_{h2}", tag="bc", bufs=2)
                nc.gpsimd.partition_broadcast(bc[:], rec[:])
                nc.vector.tensor_mul(
                    outT[hp][h2 * DK:(h2 + 1) * DK, qsl], raws[h2][:], bc[:]
                )

        window(0, 0)
        window(0, 1)
        window(1, 0)
        window(1, 1)
        for _ in gen_w4(range(SM)):
            pass

        if taps is not None:
            nc.sync.dma_start(taps["dbg_qT0"][:], qT[0][:])
            nc.sync.dma_start(taps["dbg_kT0"][:], kT[0][:])
            nc.sync.dma_start(taps["dbg_v0"][:], v_sb[0][:])
            nc.sync.dma_start(taps["dbg_outT0"][:], outT[0][:])


_NC_CACHE = None


def _get_nc():
    global _NC_CACHE
    if _NC_CACHE is None:
        _NC_CACHE = _build_kernel()
    return _NC_CACHE


def _make_in_maps(query, key, value, W1, b1, W2, b2, W3, b3, W4, b4):
    in_maps = []
    for c in range(N_CORES):
        b, g = divmod(c, 4)
        gs = slice(g * F, (g + 1) * F)
        in_maps.append({
            "xq_t": np.ascontiguousarray(query[b].T).astype(BF16),
            "xk_t": np.ascontiguousarray(key[b].T).astype(BF16),
            "xv_t": np.ascontiguousarray(value[b].T).astype(BF16),
            "w1t": np.ascontiguousarray(W1[gs, :].T).astype(BF16),
            "w2t": np.ascontiguousarray(W2[gs, :].T).astype(BF16),
            "w3t": np.ascontiguousarray(W3[gs, :].T).astype(BF16),
            "w4t": np.ascontiguousarray(W4[:, gs].T).astype(BF16),
            "b1c": np.ascontiguousarray(b1[gs].reshape(F // P, P).T).astype(np.float32),
            "b2c": np.ascontiguousarray(b2[gs].reshape(F // P, P).T).astype(np.float32),
            "b3r": b3[gs].reshape(1, F).astype(BF16),
        })
    return in_maps


def kernel(query, key, value, W1, b1, W2, b2, W3, b3, W4, b4, _trace=False, _tmpdir=None):
    args = [np.asarray(a) for a in (query, key, value, W1, b1, W2, b2, W3, b3, W4, b4)]
    nc = _get_nc()
    in_maps = _make_in_maps(*args)
    res = run_bass_kernel_spmd(
        nc, in_maps, core_ids=list(range(N_CORES)),
        trace=_trace, tmpdir=_tmpdir,
    )
    b4_f = args[10].astype(np.float32)
    full = np.zeros((B, S, D), np.float32)
    for c in range(N_CORES):
        full[c // 4] += res.results[c]["out"]
    full += b4_f[None, None, :]
    kernel.last_results = res
    return full



# revision 5
# speedup vs baseline: 1.5241x; 1.5241x over previous
"""Multi-head attention (B=2, S=2048, D=1024, H=16, d_k=64) on 8 NeuronCores.

Sharding: 8 cores = 2 batches x 4 head-groups (4 heads each).
Core c handles batch b = c//4 and heads 4*(c%4) .. 4*(c%4)+4 (feature
slice of width F=256). Each core computes its partial output-projection
contribution [S, D]; the host sums the 4 head-group partials per batch
and adds b4 (the "all-reduce" of the row-sharded W4 projection).

Device dataflow works in a "transposed world" so every matmul operand
is in its natural PE layout (contraction on partitions), with zero
on-device transposes:
  qT = W1g @ x_q.T  [F, S]   (lhsT = W1g.T host-prepped, rhs = x_q.T host-prepped)
  kT = W2g @ x_k.T  [F, S]
  v  = x_v @ W3g.T  [S, F]   (lhsT = x_v.T, rhs = W3g.T; bias via K=1 ones matmul)
  scoresT = kT_h.T @ qT_h    [S_keys, 512q x 2 heads packed]  (K=64; the two
                              head MMs are a row-tiled concurrent pair at
                              tile_position (0,0) / (64,0))
  attnT = exp(scoresT / 8)   one ACT instr per key tile, FD=1024
  pv = [v_h | ones].T @ attnT  [65, 512]; row 64 = softmax denominator
  outT_h = pv[0:64] * (1/pv[64])  (reciprocal_approx_fast + gpsimd broadcast)
  partial = outT.T @ W4g.T   [S, D]  interleaved into later windows as PE filler

Schedule: 8 windows (2 head-pairs x 4 query-quarters of 512). Within a
window, per key tile: scores pair (PE) -> exp (ACT) -> PV pair (PE),
double-buffered scores PSUM so ACT (the roofline engine at ~128us) never
starves. Input DMAs are ordered w2,xk / w3,xv / w1,xq so the k-projection
starts ~2us into the kernel instead of waiting for all 12MB of inputs.
PSUM: sc 2x2 banks + pv 2x1 + w4 2x1 = 8 banks exactly.

All matmuls bf16 with f32 PSUM accumulation.
"""

import numpy as np
import ml_dtypes

import concourse.bass as bass
import concourse.mybir as mybir
import concourse.tile as tile
from concourse import bacc
from concourse.bass_utils import run_bass_kernel_spmd

BF16 = ml_dtypes.bfloat16
F32 = mybir.dt.float32
BF = mybir.dt.bfloat16

B, S, D = 2, 2048, 1024
H_CORE = 4          # heads per core
DK = 64             # head dim
F = H_CORE * DK     # features per core = 256
P = 128             # partitions
KB = D // P         # k blocks in D contraction = 8
SM = S // P         # seq tiles of 128 = 16
QW = 512            # query window width (per head)
NQW = S // QW       # query windows = 4
N_CORES = 8
VW = H_CORE * (DK + 1)  # v with interleaved ones columns = 260


def _build_kernel():
    nc = bacc.Bacc(
        "TRN2",
        target_bir_lowering=False,
        debug=False,
        enable_asserts=False,
        num_devices=N_CORES,
    )

    xq = nc.dram_tensor("xq_t", [D, S], BF, kind="ExternalInput").ap()
    xk = nc.dram_tensor("xk_t", [D, S], BF, kind="ExternalInput").ap()
    xv = nc.dram_tensor("xv_t", [D, S], BF, kind="ExternalInput").ap()
    w1 = nc.dram_tensor("w1t", [D, F], BF, kind="ExternalInput").ap()
    w2 = nc.dram_tensor("w2t", [D, F], BF, kind="ExternalInput").ap()
    w3 = nc.dram_tensor("w3t", [D, F], BF, kind="ExternalInput").ap()
    w4 = nc.dram_tensor("w4t", [F, D], BF, kind="ExternalInput").ap()
    b1 = nc.dram_tensor("b1c", [P, F // P], F32, kind="ExternalInput").ap()
    b2 = nc.dram_tensor("b2c", [P, F // P], F32, kind="ExternalInput").ap()
    b3 = nc.dram_tensor("b3r", [1, F], BF, kind="ExternalInput").ap()
    out = nc.dram_tensor("out", [S, D], F32, kind="ExternalOutput").ap()

    with tile.TileContext(nc) as tc:
        _body(tc, xq, xk, xv, w1, w2, w3, w4, b1, b2, b3, out)

    nc.compile()
    return nc


def _body(tc, xq, xk, xv, w1, w2, w3, w4, b1, b2, b3, out):
    nc = tc.nc
    MF = F // P  # head-pair tiles in the F=256 feature dim = 2
    EXP = mybir.ActivationFunctionType.Exp

    with (
        tc.tile_pool(name="wpool", bufs=1) as wpool,
        tc.tile_pool(name="xt", bufs=1) as xt_pool,
        tc.tile_pool(name="persist", bufs=1) as persist,
        tc.tile_pool(name="attn", bufs=4) as attn_pool,
        tc.tile_pool(name="small", bufs=4) as small,
        tc.tile_pool(name="stage", bufs=2) as stage,
        tc.tile_pool(name="psum", bufs=1, space="PSUM") as psum,
    ):
        # ---- weights / holder tiles ----
        w1_sb = [wpool.tile([P, F], BF, name=f"w1_{k}", tag=f"w1_{k}") for k in range(KB)]
        w2_sb = [wpool.tile([P, F], BF, name=f"w2_{k}", tag=f"w2_{k}") for k in range(KB)]
        w3_sb = [wpool.tile([P, F], BF, name=f"w3_{k}", tag=f"w3_{k}") for k in range(KB)]
        w4_sb = [wpool.tile([P, D], BF, name=f"w4_{k}", tag=f"w4_{k}") for k in range(MF)]
        b1_sb = wpool.tile([P, MF], F32, name="b1_sb", tag="b1_sb")
        b2_sb = wpool.tile([P, MF], F32, name="b2_sb", tag="b2_sb")
        b3_sb = wpool.tile([1, F], BF, name="b3_sb", tag="b3_sb")
        ones_row = wpool.tile([1, P], BF, name="ones_row", tag="ones_row")
        nc.vector.memset(ones_row[:], 1.0)

        # persistent activations
        qT = [persist.tile([P, S], BF, name=f"qT_{m}", tag=f"qT_{m}") for m in range(MF)]
        kT = [persist.tile([P, S], BF, name=f"kT_{m}", tag=f"kT_{m}") for m in range(MF)]
        v_sb = [persist.tile([P, VW], BF, name=f"v_{s}", tag=f"v_{s}") for s in range(SM)]
        for s in range(SM):
            for h in range(H_CORE):
                nc.vector.memset(v_sb[s][:, h * (DK + 1) + DK: h * (DK + 1) + DK + 1], 1.0)
        outT = [persist.tile([P, S], BF, name=f"outT_{m}", tag=f"outT_{m}") for m in range(MF)]

        # ---- DMA order: k-proj inputs first so PE can start ~2us in ----
        def dma_w(w_sb_list, w_dram, nk):
            for k in range(nk):
                nc.sync.dma_start(w_sb_list[k][:], w_dram[k * P:(k + 1) * P, :])

        def dma_x(name, x_dram):
            ts = []
            for k in range(KB):
                t = xt_pool.tile([P, S], BF, name=f"x{name}_{k}", tag=f"xt_{name}_{k}",
                                 bufs=1)
                nc.sync.dma_start(t[:], x_dram[k * P:(k + 1) * P, :])
                ts.append(t)
            return ts

        dma_w(w2_sb, w2, KB)
        nc.sync.dma_start(b2_sb[:], b2[:])
        xk_sb = dma_x("k", xk)
        dma_w(w3_sb, w3, KB)
        nc.sync.dma_start(b3_sb[:], b3[:])
        xv_sb = dma_x("v", xv)
        dma_w(w1_sb, w1, KB)
        nc.sync.dma_start(b1_sb[:], b1[:])
        xq_sb = dma_x("q", xq)
        dma_w(w4_sb, w4, MF)

        # ---- q/k projections: dst[m][:, quarter] = sum_k W[k][:,m].T @ x[k][:, quarter]
        #      weights held stationary across the 4 query quarters ----
        def proj_qk(x_sb, w_sb, b_sb, dst):
            for m in range(MF):
                ps = [psum.tile([P, 1024], F32, name=f"pp_{m}_{i}", tag="sc", bufs=2)
                      for i in range(2)]
                for k in range(KB):
                    for qq in range(4):
                        nc.tensor.matmul(
                            ps[qq // 2][:, (qq % 2) * 512:(qq % 2 + 1) * 512],
                            w_sb[k][:, m * P:(m + 1) * P],
                            x_sb[k][:, qq * 512:(qq + 1) * 512],
                            start=(k == 0),
                            stop=(k == KB - 1),
                        )
                for i in range(2):
                    nc.vector.tensor_scalar_add(
                        dst[m][:, i * 1024:(i + 1) * 1024], ps[i][:], b_sb[:, m:m + 1]
                    )

        proj_qk(xk_sb, w2_sb, b2_sb, kT)

        # ---- v projection (natural layout): v[s] = xv[:, s].T @ W3t + b3 ----
        for s in range(SM):
            ps = psum.tile([P, F], F32, name=f"pv_{s}", tag="pv", bufs=2)
            for k in range(KB):
                nc.tensor.matmul(
                    ps[:],
                    xv_sb[k][:, s * P:(s + 1) * P],
                    w3_sb[k][:],
                    start=(k == 0),
                    stop=False,
                )
            nc.tensor.matmul(ps[:], ones_row[:], b3_sb[:], start=False, stop=True)
            for h in range(H_CORE):
                nc.vector.tensor_copy(
                    v_sb[s][:, h * (DK + 1): h * (DK + 1) + DK],
                    ps[:, h * DK:(h + 1) * DK],
                )

        proj_qk(xq_sb, w1_sb, b1_sb, qT)

        # ---- output projection generator: used as PE filler inside windows ----
        def gen_w4(qts):
            for qt in qts:
                ob = stage.tile([P, D], F32, name=f"ob_{qt}", tag="ob", bufs=2)
                for oc in range(2):
                    ps = psum.tile([P, 512], F32, name=f"po_{qt}_{oc}", tag="w4", bufs=2)
                    for m in range(MF):
                        nc.tensor.matmul(
                            ps[:],
                            outT[m][:, qt * P:(qt + 1) * P],
                            w4_sb[m][:, oc * 512:(oc + 1) * 512],
                            start=(m == 0),
                            stop=(m == MF - 1),
                        )
                    nc.vector.tensor_copy(ob[:, oc * 512:(oc + 1) * 512], ps[:])
                    yield
                nc.sync.dma_start(out[qt * P:(qt + 1) * P, :], ob[:])
                yield

        # ---- attention window: head-pair hp, query quarter qw (512 queries).
        #      Both heads' scores packed in one [128, 1024] PSUM tile:
        #      cols 0:512 = head 2hp (K-rows 0:64), cols 512:1024 = head 2hp+1
        #      (K-rows 64:128) -> row-tiled concurrent MM pair, single exp. ----
        def window(hp, qw, filler=None):
            qsl = slice(qw * QW, (qw + 1) * QW)
            attn_t = [None] * SM
            pv_ps = [
                psum.tile([DK + 1, QW], F32, name=f"pvps_{hp}_{qw}_{h2}", tag="pv", bufs=2)
                for h2 in range(2)
            ]

            def emit_scores(kt):
                sc = psum.tile([P, 1024], F32, name=f"sc_{hp}_{qw}_{kt}", tag="sc", bufs=2)
                for h2 in range(2):
                    rsl = slice(h2 * DK, (h2 + 1) * DK)
                    nc.tensor.matmul(
                        sc[:, h2 * 512:(h2 + 1) * 512],
                        kT[hp][rsl, kt * P:(kt + 1) * P],
                        qT[hp][rsl, qsl],
                        start=True,
                        stop=True,
                    )
                at = attn_pool.tile([P, 1024], BF, name=f"at_{hp}_{qw}_{kt}",
                                    tag="attnT", bufs=4)
                nc.scalar.activation(
                    at[:], sc[:], EXP, scale=1.0 / np.sqrt(DK),
                )
                attn_t[kt] = at

            def emit_pv(kt):
                for h2 in range(2):
                    h = hp * 2 + h2
                    vsl = slice(h * (DK + 1), h * (DK + 1) + DK + 1)
                    nc.tensor.matmul(
                        pv_ps[h2][:],
                        v_sb[kt][:, vsl],
                        attn_t[kt][:, h2 * 512:(h2 + 1) * 512],
                        start=(kt == 0),
                        stop=(kt == SM - 1),
                    )

            emit_scores(0)
            for kt in range(1, SM):
                emit_scores(kt)
                emit_pv(kt - 1)
                if filler is not None and kt >= 3:
                    next(filler, None)
            emit_pv(SM - 1)

            # normalize: den row 64 -> recip -> broadcast -> scale, straight
            # from PSUM (pv bufs=2 so next window's PV is not blocked).
            for h2 in range(2):
                den = small.tile([1, QW], F32, name=f"den_{hp}_{qw}_{h2}", tag="den", bufs=3)
                nc.vector.tensor_copy(den[:], pv_ps[h2][DK:DK + 1, :])
                rec = small.tile([1, QW], F32, name=f"rec_{hp}_{qw}_{h2}", tag="rec", bufs=3)
                nc.vector.reciprocal_approx_fast(rec[:], den[:])
                bc = small.tile([DK, QW], F32, name=f"bc_{hp}_{qw}_{h2}", tag="bc", bufs=2)
                nc.gpsimd.partition_broadcast(bc[:], rec[:])
                nc.vector.tensor_mul(
                    outT[hp][h2 * DK:(h2 + 1) * DK, qsl], pv_ps[h2][0:DK, :], bc[:]
                )

        # window order: qw outer so each query-quarter finishes across both
        # head-pairs, then its W4 qtiles fill the next quarter's windows.
        fill = iter(())
        for qw in range(NQW):
            window(0, qw, fill)
            window(1, qw, fill)
            fill = gen_w4(range(qw * 4, (qw + 1) * 4))
        for _ in fill:
            pass


_NC_CACHE = None


def _get_nc():
    global _NC_CACHE
    if _NC_CACHE is None:
        _NC_CACHE = _build_kernel()
    return _NC_CACHE


def _make_in_maps(query, key, value, W1, b1, W2, b2, W3, b3, W4, b4):
    in_maps = []
    for c in range(N_CORES):
        b, g = divmod(c, 4)
        gs = slice(g * F, (g + 1) * F)
        in_maps.append({
            "xq_t": np.ascontiguousarray(query[b].T).astype(BF16),
            "xk_t": np.ascontiguousarray(key[b].T).astype(BF16),
            "xv_t": np.ascontiguousarray(value[b].T).astype(BF16),
            "w1t": np.ascontiguousarray(W1[gs, :].T).astype(BF16),
            "w2t": np.ascontiguousarray(W2[gs, :].T).astype(BF16),
            "w3t": np.ascontiguousarray(W3[gs, :].T).astype(BF16),
            "w4t": np.ascontiguousarray(W4[:, gs].T).astype(BF16),
            "b1c": np.ascontiguousarray(b1[gs].reshape(F // P, P).T).astype(np.float32),
            "b2c": np.ascontiguousarray(b2[gs].reshape(F // P, P).T).astype(np.float32),
            "b3r": b3[gs].reshape(1, F).astype(BF16),
        })
    return in_maps


def kernel(query, key, value, W1, b1, W2, b2, W3, b3, W4, b4, _trace=False, _tmpdir=None):
    args = [np.asarray(a) for a in (query, key, value, W1, b1, W2, b2, W3, b3, W4, b4)]
    nc = _get_nc()
    in_maps = _make_in_maps(*args)
    res = run_bass_kernel_spmd(
        nc, in_maps, core_ids=list(range(N_CORES)),
        trace=_trace, tmpdir=_tmpdir,
    )
    b4_f = args[10].astype(np.float32)
    full = np.zeros((B, S, D), np.float32)
    for c in range(N_CORES):
        full[c // 4] += res.results[c]["out"]
    full += b4_f[None, None, :]
    kernel.last_results = res
    return full


# revision 8
# speedup vs baseline: 1.5897x; 1.0430x over previous
"""Multi-head attention (B=2, S=2048, D=1024, H=16, d_k=64) on 8 NeuronCores.

Sharding: 8 cores = 2 batches x 4 head-groups (4 heads each).
Core c handles batch b = c//4 and heads 4*(c%4) .. 4*(c%4)+4 (feature
slice of width F=256). Each core computes its partial output-projection
contribution [S, D] in bf16; the host sums the 4 head-group partials per
batch in f32 and adds b4.

Device dataflow ("transposed world", zero-layout-change matmuls):
  qT = W1g @ x_q.T  [F, S]      kT = W2g @ x_k.T  [F, S]
  vT = W3g @ x_v.T  [F, S]  -> PE-transposed per 128-block into
  v   [S, F] with interleaved ones columns (softmax denominator trick)
  scoresT = kT_h.T @ qT_h  [S_keys, 512q x 2 heads packed]  (K=64; the two
            head MMs are a row-tiled concurrent pair, tile_position (0,0)/(64,0))
  attnT = exp(scoresT / 8)  one ACT instr per key tile, FD=1024
  pv = [v_h | ones].T @ attnT  [65, 512]; row 64 = denominator
  outT_h = pv[0:64] * recip(pv[64])  (reciprocal_approx_fast + gpsimd bcast)
  partial = outT.T @ W4g.T  [S, D] interleaved into later windows as PE filler

Schedule: the 8 attention windows (2 head-pairs x 4 query-quarters) run as
ONE flat software-pipelined stream over 128 (window, key-tile) steps:
scores(j+1) is emitted before pv(j), so the PE keeps streaming across
window boundaries and the ScalarE exp pipe (the ~140us roofline engine)
never gaps. DMA order xk | xv | xq so each projection starts as its
inputs land. PSUM: sc 2x2 banks + pv 2x1 + w4 2x1 = 8 banks exactly.
"""

import numpy as np
import ml_dtypes

import concourse.bass as bass
import concourse.mybir as mybir
import concourse.tile as tile
from concourse import bacc
from concourse.bass_utils import run_bass_kernel_spmd

BF16 = ml_dtypes.bfloat16
F32 = mybir.dt.float32
BF = mybir.dt.bfloat16

B, S, D = 2, 2048, 1024
H_CORE = 4          # heads per core
DK = 64             # head dim
F = H_CORE * DK     # features per core = 256
P = 128             # partitions
KB = D // P         # k blocks in D contraction = 8
SM = S // P         # seq tiles of 128 = 16
QW = 512            # query window width (per head)
NQW = S // QW       # query quarters = 4
N_CORES = 8
VW = H_CORE * (DK + 1)  # v with interleaved ones columns = 260


def _build_kernel():
    nc = bacc.Bacc(
        "TRN2",
        target_bir_lowering=False,
        debug=False,
        enable_asserts=False,
        num_devices=N_CORES,
    )

    xq = nc.dram_tensor("xq_t", [D, S], BF, kind="ExternalInput").ap()
    xk = nc.dram_tensor("xk_t", [D, S], BF, kind="ExternalInput").ap()
    xv = nc.dram_tensor("xv_t", [D, S], BF, kind="ExternalInput").ap()
    w1 = nc.dram_tensor("w1t", [D, F], BF, kind="ExternalInput").ap()
    w2 = nc.dram_tensor("w2t", [D, F], BF, kind="ExternalInput").ap()
    w3 = nc.dram_tensor("w3t", [D, F], BF, kind="ExternalInput").ap()
    w4 = nc.dram_tensor("w4t", [F, D], BF, kind="ExternalInput").ap()
    b1 = nc.dram_tensor("b1c", [P, F // P], F32, kind="ExternalInput").ap()
    b2 = nc.dram_tensor("b2c", [P, F // P], F32, kind="ExternalInput").ap()
    b3 = nc.dram_tensor("b3c", [P, F // P], F32, kind="ExternalInput").ap()
    ident = nc.dram_tensor("ident", [P, P], BF, kind="ExternalInput").ap()
    out = nc.dram_tensor("out", [S, D], BF, kind="ExternalOutput").ap()

    with tile.TileContext(nc) as tc:
        _body(tc, xq, xk, xv, w1, w2, w3, w4, b1, b2, b3, ident, out)

    nc.compile()
    return nc


def _body(tc, xq, xk, xv, w1, w2, w3, w4, b1, b2, b3, ident, out):
    nc = tc.nc
    MF = F // P  # head-pair tiles in the F=256 feature dim = 2
    EXP = mybir.ActivationFunctionType.Exp

    with (
        tc.tile_pool(name="wpool", bufs=1) as wpool,
        tc.tile_pool(name="xt", bufs=1) as xt_pool,
        tc.tile_pool(name="persist", bufs=1) as persist,
        tc.tile_pool(name="attn", bufs=6) as attn_pool,
        tc.tile_pool(name="small", bufs=4) as small,
        tc.tile_pool(name="stage", bufs=2) as stage,
        tc.tile_pool(name="psum", bufs=1, space="PSUM") as psum,
    ):
        # ---- weight / constant holder tiles ----
        w1_sb = [wpool.tile([P, F], BF, name=f"w1_{k}", tag=f"w1_{k}") for k in range(KB)]
        w2_sb = [wpool.tile([P, F], BF, name=f"w2_{k}", tag=f"w2_{k}") for k in range(KB)]
        w3_sb = [wpool.tile([P, F], BF, name=f"w3_{k}", tag=f"w3_{k}") for k in range(KB)]
        w4_sb = [wpool.tile([P, D], BF, name=f"w4_{k}", tag=f"w4_{k}") for k in range(MF)]
        b1_sb = wpool.tile([P, MF], F32, name="b1_sb", tag="b1_sb")
        b2_sb = wpool.tile([P, MF], F32, name="b2_sb", tag="b2_sb")
        b3_sb = wpool.tile([P, MF], F32, name="b3_sb", tag="b3_sb")
        id_sb = wpool.tile([P, P], BF, name="id_sb", tag="id_sb")

        # persistent activations
        qT = [persist.tile([P, S], BF, name=f"qT_{m}", tag=f"qT_{m}") for m in range(MF)]
        kT = [persist.tile([P, S], BF, name=f"kT_{m}", tag=f"kT_{m}") for m in range(MF)]
        vT = [persist.tile([P, S], BF, name=f"vT_{m}", tag=f"vT_{m}") for m in range(MF)]
        v_sb = [persist.tile([P, VW], BF, name=f"v_{s}", tag=f"v_{s}") for s in range(SM)]
        for s in range(SM):
            for h in range(H_CORE):
                nc.vector.memset(v_sb[s][:, h * (DK + 1) + DK: h * (DK + 1) + DK + 1], 1.0)
        outT = [persist.tile([P, S], BF, name=f"outT_{m}", tag=f"outT_{m}") for m in range(MF)]

        # ---- DMA order: k inputs, then v, then q, so each projection
        #      starts as soon as its tiles land ----
        def dma_w(w_sb_list, w_dram, nk):
            for k in range(nk):
                nc.sync.dma_start(w_sb_list[k][:], w_dram[k * P:(k + 1) * P, :])

        def dma_x(name, x_dram):
            ts = []
            for k in range(KB):
                t = xt_pool.tile([P, S], BF, name=f"x{name}_{k}", tag=f"xt_{name}_{k}",
                                 bufs=1)
                nc.sync.dma_start(t[:], x_dram[k * P:(k + 1) * P, :])
                ts.append(t)
            return ts

        dma_w(w2_sb, w2, KB)
        nc.sync.dma_start(b2_sb[:], b2[:])
        xk_sb = dma_x("k", xk)
        dma_w(w3_sb, w3, KB)
        nc.sync.dma_start(b3_sb[:], b3[:])
        nc.sync.dma_start(id_sb[:], ident[:])
        xv_sb = dma_x("v", xv)
        dma_w(w1_sb, w1, KB)
        nc.sync.dma_start(b1_sb[:], b1[:])
        xq_sb = dma_x("q", xq)
        dma_w(w4_sb, w4, MF)

        # ---- projections to transposed layout [F-slice, S] ----
        def proj_qk(x_sb, w_sb, b_sb, dst):
            for m in range(MF):
                ps = [psum.tile([P, 1024], F32, name=f"pp_{m}_{i}", tag="sc", bufs=2)
                      for i in range(2)]
                for k in range(KB):
                    for qq in range(4):
                        nc.tensor.matmul(
                            ps[qq // 2][:, (qq % 2) * 512:(qq % 2 + 1) * 512],
                            w_sb[k][:, m * P:(m + 1) * P],
                            x_sb[k][:, qq * 512:(qq + 1) * 512],
                            start=(k == 0),
                            stop=(k == KB - 1),
                        )
                for i in range(2):
                    nc.vector.tensor_scalar_add(
                        dst[m][:, i * 1024:(i + 1) * 1024], ps[i][:], b_sb[:, m:m + 1]
                    )

        proj_qk(xk_sb, w2_sb, b2_sb, kT)
        proj_qk(xv_sb, w3_sb, b3_sb, vT)

        # vT -> v: PE transpose each [128,128] block, then cast the two
        # 64-wide head slices into v_sb around the ones columns.
        for s in range(SM):
            for m in range(MF):
                tp = psum.tile([P, P], BF, name=f"tp_{s}_{m}", tag="w4", bufs=2)
                nc.tensor.transpose(tp[:], vT[m][:, s * P:(s + 1) * P], id_sb[:])
                for hh in range(2):
                    h = m * 2 + hh
                    nc.vector.tensor_copy(
                        v_sb[s][:, h * (DK + 1): h * (DK + 1) + DK],
                        tp[:, hh * DK:(hh + 1) * DK],
                    )

        proj_qk(xq_sb, w1_sb, b1_sb, qT)

        # ---- output projection generator (PE filler inside windows) ----
        def gen_w4(qts):
            for qt in qts:
                ob = stage.tile([P, D], BF, name=f"ob_{qt}", tag="ob", bufs=2)
                for oc in range(2):
                    ps = psum.tile([P, 512], F32, name=f"po_{qt}_{oc}", tag="w4", bufs=2)
                    for m in range(MF):
                        nc.tensor.matmul(
                            ps[:],
                            outT[m][:, qt * P:(qt + 1) * P],
                            w4_sb[m][:, oc * 512:(oc + 1) * 512],
                            start=(m == 0),
                            stop=(m == MF - 1),
                        )
                    nc.vector.tensor_copy(ob[:, oc * 512:(oc + 1) * 512], ps[:])
                    yield
                nc.sync.dma_start(out[qt * P:(qt + 1) * P, :], ob[:])
                yield

        # ---- attention: one flat pipelined stream over all 8 windows ----
        wins = [(hp, qw) for qw in range(NQW) for hp in range(MF)]
        NW = len(wins)
        pv_ps = {}      # (w_i, h2) -> psum tile
        attn_t = {}     # j -> attn tile

        def emit_scores(j):
            w_i, kt = divmod(j, SM)
            hp, qw = wins[w_i]
            sc = psum.tile([P, 1024], F32, name=f"sc_{j}", tag="sc", bufs=2)
            for h2 in range(2):
                rsl = slice(h2 * DK, (h2 + 1) * DK)
                nc.tensor.matmul(
                    sc[:, h2 * 512:(h2 + 1) * 512],
                    kT[hp][rsl, kt * P:(kt + 1) * P],
                    qT[hp][rsl, qw * QW:(qw + 1) * QW],
                    start=True,
                    stop=True,
                )
            at = attn_pool.tile([P, 1024], BF, name=f"at_{j}", tag="attnT", bufs=6)
            nc.scalar.activation(at[:], sc[:], EXP, scale=1.0 / np.sqrt(DK))
            attn_t[j] = at

        def emit_pv(j):
            w_i, kt = divmod(j, SM)
            hp, qw = wins[w_i]
            if kt == 0:
                for h2 in range(2):
                    pv_ps[(w_i, h2)] = psum.tile(
                        [DK + 1, QW], F32, name=f"pvps_{w_i}_{h2}", tag="pv", bufs=2)
            for h2 in range(2):
                h = hp * 2 + h2
                vsl = slice(h * (DK + 1), h * (DK + 1) + DK + 1)
                nc.tensor.matmul(
                    pv_ps[(w_i, h2)][:],
                    v_sb[kt][:, vsl],
                    attn_t[j][:, h2 * 512:(h2 + 1) * 512],
                    start=(kt == 0),
                    stop=(kt == SM - 1),
                )
            del attn_t[j]

        def emit_norm(w_i):
            hp, qw = wins[w_i]
            qsl = slice(qw * QW, (qw + 1) * QW)
            dens, raws = [], []
            # fast PSUM release: pull den (f32) + raw outT (bf16) out of the
            # pv banks first so the next window's PV accumulation can start.
            for h2 in range(2):
                pv = pv_ps.pop((w_i, h2))
                den = small.tile([1, QW], F32, name=f"den_{w_i}_{h2}", tag="den", bufs=3)
                nc.vector.tensor_copy(den[:], pv[DK:DK + 1, :])
                raw = small.tile([DK, QW], BF, name=f"raw_{w_i}_{h2}", tag="raw", bufs=3)
                nc.vector.tensor_copy(raw[:], pv[0:DK, :])
                dens.append(den)
                raws.append(raw)
            for h2 in range(2):
                rec = small.tile([1, QW], F32, name=f"rec_{w_i}_{h2}", tag="rec", bufs=3)
                nc.vector.reciprocal_approx_fast(rec[:], dens[h2][:])
                bc = small.tile([DK, QW], F32, name=f"bc_{w_i}_{h2}", tag="bc", bufs=2)
                nc.gpsimd.partition_broadcast(bc[:], rec[:])
                nc.vector.tensor_mul(
                    outT[hp][h2 * DK:(h2 + 1) * DK, qsl], raws[h2][:], bc[:]
                )

        NSTEP = NW * SM
        fill = iter(())
        emit_scores(0)
        for j in range(1, NSTEP):
            emit_scores(j)
            emit_pv(j - 1)
            w_prev, kt_prev = divmod(j - 1, SM)
            if kt_prev == SM - 1:
                emit_norm(w_prev)
                hp_p, qw_p = wins[w_prev]
                if hp_p == MF - 1:  # quarter qw_p fully done -> queue its W4
                    fill = gen_w4(range(qw_p * 4, (qw_p + 1) * 4))
            next(fill, None)
        emit_pv(NSTEP - 1)
        emit_norm(NW - 1)
        for _ in fill:
            pass
        for _ in gen_w4(range((NQW - 1) * 4, NQW * 4)):
            pass


_NC_CACHE = None


def _get_nc():
    global _NC_CACHE
    if _NC_CACHE is None:
        _NC_CACHE = _build_kernel()
    return _NC_CACHE


def _make_in_maps(query, key, value, W1, b1, W2, b2, W3, b3, W4, b4):
    in_maps = []
    ident = np.eye(P, dtype=BF16)
    for c in range(N_CORES):
        b, g = divmod(c, 4)
        gs = slice(g * F, (g + 1) * F)
        in_maps.append({
            "xq_t": np.ascontiguousarray(query[b].T).astype(BF16),
            "xk_t": np.ascontiguousarray(key[b].T).astype(BF16),
            "xv_t": np.ascontiguousarray(value[b].T).astype(BF16),
            "w1t": np.ascontiguousarray(W1[gs, :].T).astype(BF16),
            "w2t": np.ascontiguousarray(W2[gs, :].T).astype(BF16),
            "w3t": np.ascontiguousarray(W3[gs, :].T).astype(BF16),
            "w4t": np.ascontiguousarray(W4[:, gs].T).astype(BF16),
            "b1c": np.ascontiguousarray(b1[gs].reshape(F // P, P).T).astype(np.float32),
            "b2c": np.ascontiguousarray(b2[gs].reshape(F // P, P).T).astype(np.float32),
            "b3c": np.ascontiguousarray(b3[gs].reshape(F // P, P).T).astype(np.float32),
            "ident": ident,
        })
    return in_maps


def kernel(query, key, value, W1, b1, W2, b2, W3, b3, W4, b4, _trace=False, _tmpdir=None):
    args = [np.asarray(a) for a in (query, key, value, W1, b1, W2, b2, W3, b3, W4, b4)]
    nc = _get_nc()
    in_maps = _make_in_maps(*args)
    res = run_bass_kernel_spmd(
        nc, in_maps, core_ids=list(range(N_CORES)),
        trace=_trace, tmpdir=_tmpdir,
    )
    b4_f = args[10].astype(np.float32)
    full = np.zeros((B, S, D), np.float32)
    for c in range(N_CORES):
        full[c // 4] += res.results[c]["out"].astype(np.float32)
    full += b4_f[None, None, :]
    kernel.last_results = res
    return full


# revision 12
# speedup vs baseline: 1.6416x; 1.0326x over previous
"""Multi-head attention (B=2, S=2048, D=1024, H=16, d_k=64) on 8 NeuronCores.

Sharding: 8 cores = 2 batches x 4 head-groups (4 heads each).
Core c handles batch b = c//4 and heads 4*(c%4) .. 4*(c%4)+4 (feature
slice of width F=256). Each core computes its partial output-projection
contribution [S, D] in bf16; the host sums the 4 head-group partials per
batch in f32 and adds b4.

Device dataflow ("transposed world", zero-layout-change matmuls):
  qT = W1g @ x_q.T  [F, S]      kT = W2g @ x_k.T  [F, S]
  vT = W3g @ x_v.T  [F, S]  -> PE-transposed per 128-block into
  v   [S, F] with interleaved ones columns (softmax denominator trick)
  scoresT = kT_h.T @ qT_h  [S_keys, 512q x 2 heads packed]  (K=64; the two
            head MMs are a row-tiled concurrent pair, tile_position (0,0)/(64,0))
  attnT = exp(scoresT / 8)  one ACT instr per key tile, FD=1024
  pv = [v_h | ones].T @ attnT  [65, 512]; row 64 = denominator
  outT_h = pv[0:64] * recip(pv[64])  (reciprocal_approx_fast + gpsimd bcast)
  partial = outT.T @ W4g.T  [S, D] interleaved into later windows as PE filler

Schedule: the 8 attention windows (2 head-pairs x 4 query-quarters) run as
ONE flat software-pipelined stream over 128 (window, key-tile) steps:
scores(j+1) is emitted before pv(j), so the PE keeps streaming across
window boundaries and the ScalarE exp pipe (the ~140us roofline engine)
never gaps. DMA order xk | xv | xq so each projection starts as its
inputs land. PSUM: sc 2x2 banks + pv 2x1 + w4 2x1 = 8 banks exactly.
"""

import numpy as np
import ml_dtypes

import concourse.bass as bass
import concourse.mybir as mybir
import concourse.tile as tile
from concourse import bacc
from concourse.bass_utils import run_bass_kernel_spmd

BF16 = ml_dtypes.bfloat16
F32 = mybir.dt.float32
BF = mybir.dt.bfloat16

B, S, D = 2, 2048, 1024
H_CORE = 4          # heads per core
DK = 64             # head dim
F = H_CORE * DK     # features per core = 256
P = 128             # partitions
KB = D // P         # k blocks in D contraction = 8
SM = S // P         # seq tiles of 128 = 16
QW = 512            # query window width (per head)
NQW = S // QW       # query quarters = 4
N_CORES = 8
VW = H_CORE * (DK + 1)  # v with interleaved ones columns = 260


def _build_kernel():
    nc = bacc.Bacc(
        "TRN2",
        target_bir_lowering=False,
        debug=False,
        enable_asserts=False,
        num_devices=N_CORES,
    )

    xq = nc.dram_tensor("xq_t", [D, S], BF, kind="ExternalInput").ap()
    xk = nc.dram_tensor("xk_t", [D, S], BF, kind="ExternalInput").ap()
    xv = nc.dram_tensor("xv_t", [D, S], BF, kind="ExternalInput").ap()
    w1 = nc.dram_tensor("w1t", [D, F], BF, kind="ExternalInput").ap()
    w2 = nc.dram_tensor("w2t", [D, F], BF, kind="ExternalInput").ap()
    w3 = nc.dram_tensor("w3t", [D, F], BF, kind="ExternalInput").ap()
    w4 = nc.dram_tensor("w4t", [F, D], BF, kind="ExternalInput").ap()
    b1 = nc.dram_tensor("b1c", [P, F // P], F32, kind="ExternalInput").ap()
    b2 = nc.dram_tensor("b2c", [P, F // P], F32, kind="ExternalInput").ap()
    b3 = nc.dram_tensor("b3c", [P, F // P], F32, kind="ExternalInput").ap()
    ident = nc.dram_tensor("ident", [P, P], BF, kind="ExternalInput").ap()
    out = nc.dram_tensor("out", [S, D], BF, kind="ExternalOutput").ap()

    with tile.TileContext(nc) as tc:
        _body(tc, xq, xk, xv, w1, w2, w3, w4, b1, b2, b3, ident, out)

    nc.compile()
    return nc


def _body(tc, xq, xk, xv, w1, w2, w3, w4, b1, b2, b3, ident, out):
    nc = tc.nc
    MF = F // P  # head-pair tiles in the F=256 feature dim = 2
    EXP = mybir.ActivationFunctionType.Exp

    with (
        tc.tile_pool(name="wpool", bufs=1) as wpool,
        tc.tile_pool(name="xt", bufs=1) as xt_pool,
        tc.tile_pool(name="persist", bufs=1) as persist,
        tc.tile_pool(name="attn", bufs=6) as attn_pool,
        tc.tile_pool(name="small", bufs=4) as small,
        tc.tile_pool(name="stage", bufs=2) as stage,
        tc.tile_pool(name="psum", bufs=1, space="PSUM") as psum,
    ):
        # ---- weight / constant holder tiles ----
        w1_sb = [wpool.tile([P, F], BF, name=f"w1_{k}", tag=f"w1_{k}") for k in range(KB)]
        w2_sb = [wpool.tile([P, F], BF, name=f"w2_{k}", tag=f"w2_{k}") for k in range(KB)]
        w3_sb = [wpool.tile([P, F], BF, name=f"w3_{k}", tag=f"w3_{k}") for k in range(KB)]
        w4_sb = [wpool.tile([P, D], BF, name=f"w4_{k}", tag=f"w4_{k}") for k in range(MF)]
        b1_sb = wpool.tile([P, MF], F32, name="b1_sb", tag="b1_sb")
        b2_sb = wpool.tile([P, MF], F32, name="b2_sb", tag="b2_sb")
        b3_sb = wpool.tile([P, MF], F32, name="b3_sb", tag="b3_sb")
        id_sb = wpool.tile([P, P], BF, name="id_sb", tag="id_sb")

        # persistent activations
        qT = [persist.tile([P, S], BF, name=f"qT_{m}", tag=f"qT_{m}") for m in range(MF)]
        kT = [persist.tile([P, S], BF, name=f"kT_{m}", tag=f"kT_{m}") for m in range(MF)]
        vT = [persist.tile([P, S], BF, name=f"vT_{m}", tag=f"vT_{m}") for m in range(MF)]
        v_sb = [persist.tile([P, VW], BF, name=f"v_{s}", tag=f"v_{s}") for s in range(SM)]
        for s in range(SM):
            for h in range(H_CORE):
                nc.vector.memset(v_sb[s][:, h * (DK + 1) + DK: h * (DK + 1) + DK + 1], 1.0)
        outT = [persist.tile([P, S], BF, name=f"outT_{m}", tag=f"outT_{m}") for m in range(MF)]

        # ---- DMA order: k inputs, then v, then q, so each projection
        #      starts as soon as its tiles land ----
        def dma_w(w_sb_list, w_dram, nk):
            for k in range(nk):
                nc.sync.dma_start(w_sb_list[k][:], w_dram[k * P:(k + 1) * P, :])

        def dma_x(name, x_dram):
            ts = []
            for k in range(KB):
                t = xt_pool.tile([P, S], BF, name=f"x{name}_{k}", tag=f"xt_{name}_{k}",
                                 bufs=1)
                nc.sync.dma_start(t[:], x_dram[k * P:(k + 1) * P, :])
                ts.append(t)
            return ts

        dma_w(w2_sb, w2, KB)
        nc.sync.dma_start(b2_sb[:], b2[:])
        xk_sb = dma_x("k", xk)
        dma_w(w3_sb, w3, KB)
        nc.sync.dma_start(b3_sb[:], b3[:])
        nc.sync.dma_start(id_sb[:], ident[:])
        xv_sb = dma_x("v", xv)
        dma_w(w1_sb, w1, KB)
        nc.sync.dma_start(b1_sb[:], b1[:])
        xq_sb = dma_x("q", xq)
        dma_w(w4_sb, w4, MF)

        # ---- projections to transposed layout [F-slice, S] ----
        # The 4 query-quarter MMs per (m, k) share the same stationary weights;
        # skip the redundant LDWEIGHTS on the last 3. Bias rides on ScalarE
        # (idle during projections) instead of the DVE.
        IDENT = mybir.ActivationFunctionType.Identity

        def proj_qk(x_sb, w_sb, b_sb, dst):
            for m in range(MF):
                ps = [psum.tile([P, 1024], F32, name=f"pp_{m}_{i}", tag="sc", bufs=2)
                      for i in range(2)]
                for k in range(KB):
                    for qq in range(4):
                        mm = nc.tensor.matmul(
                            ps[qq // 2][:, (qq % 2) * 512:(qq % 2 + 1) * 512],
                            w_sb[k][:, m * P:(m + 1) * P],
                            x_sb[k][:, qq * 512:(qq + 1) * 512],
                            start=(k == 0),
                            stop=(k == KB - 1),
                        )
                        if qq > 0:
                            mm.ins.ldweights = False
                for i in range(2):
                    nc.scalar.activation(
                        dst[m][:, i * 1024:(i + 1) * 1024], ps[i][:], IDENT,
                        bias=b_sb[:, m:m + 1],
                    )

        proj_qk(xk_sb, w2_sb, b2_sb, kT)
        proj_qk(xv_sb, w3_sb, b3_sb, vT)

        # vT -> v: PE transpose each [128,128] block, then cast the two
        # 64-wide head slices into v_sb around the ones columns.
        for s in range(SM):
            for m in range(MF):
                tp = psum.tile([P, P], BF, name=f"tp_{s}_{m}", tag="w4", bufs=2)
                nc.tensor.transpose(tp[:], vT[m][:, s * P:(s + 1) * P], id_sb[:])
                for hh in range(2):
                    h = m * 2 + hh
                    nc.vector.tensor_copy(
                        v_sb[s][:, h * (DK + 1): h * (DK + 1) + DK],
                        tp[:, hh * DK:(hh + 1) * DK],
                    )

        proj_qk(xq_sb, w1_sb, b1_sb, qT)

        # ---- output projection generator (PE filler inside windows).
        #      tail=True routes the PSUM->SBUF copies to ScalarE (idle once
        #      the exps are done) to shorten the final-quarter tail. ----
        def gen_w4(qts, tail=False):
            for qt in qts:
                ob = stage.tile([P, D], BF, name=f"ob_{qt}", tag="ob", bufs=2)
                for oc in range(2):
                    ps = psum.tile([P, 512], F32, name=f"po_{qt}_{oc}", tag="w4", bufs=2)
                    for m in range(MF):
                        nc.tensor.matmul(
                            ps[:],
                            outT[m][:, qt * P:(qt + 1) * P],
                            w4_sb[m][:, oc * 512:(oc + 1) * 512],
                            start=(m == 0),
                            stop=(m == MF - 1),
                        )
                    if tail:
                        nc.scalar.copy(ob[:, oc * 512:(oc + 1) * 512], ps[:])
                    else:
                        nc.vector.tensor_copy(ob[:, oc * 512:(oc + 1) * 512], ps[:])
                    yield
                nc.sync.dma_start(out[qt * P:(qt + 1) * P, :], ob[:])
                yield

        # ---- attention: one flat pipelined stream over all 8 windows ----
        wins = [(hp, qw) for qw in range(NQW) for hp in range(MF)]
        NW = len(wins)
        pv_ps = {}      # (w_i, h2) -> psum tile
        attn_t = {}     # j -> attn tile

        def emit_scores(j):
            w_i, kt = divmod(j, SM)
            hp, qw = wins[w_i]
            sc = psum.tile([P, 1024], F32, name=f"sc_{j}", tag="sc", bufs=2)
            for h2 in range(2):
                rsl = slice(h2 * DK, (h2 + 1) * DK)
                nc.tensor.matmul(
                    sc[:, h2 * 512:(h2 + 1) * 512],
                    kT[hp][rsl, kt * P:(kt + 1) * P],
                    qT[hp][rsl, qw * QW:(qw + 1) * QW],
                    start=True,
                    stop=True,
                )
            at = attn_pool.tile([P, 1024], BF, name=f"at_{j}", tag="attnT", bufs=6)
            nc.scalar.activation(at[:], sc[:], EXP, scale=1.0 / np.sqrt(DK))
            attn_t[j] = at

        def emit_pv(j):
            w_i, kt = divmod(j, SM)
            hp, qw = wins[w_i]
            if kt == 0:
                for h2 in range(2):
                    pv_ps[(w_i, h2)] = psum.tile(
                        [DK + 1, QW], F32, name=f"pvps_{w_i}_{h2}", tag="pv", bufs=2)
            for h2 in range(2):
                h = hp * 2 + h2
                vsl = slice(h * (DK + 1), h * (DK + 1) + DK + 1)
                nc.tensor.matmul(
                    pv_ps[(w_i, h2)][:],
                    v_sb[kt][:, vsl],
                    attn_t[j][:, h2 * 512:(h2 + 1) * 512],
                    start=(kt == 0),
                    stop=(kt == SM - 1),
                )
            del attn_t[j]

        def emit_norm(w_i):
            hp, qw = wins[w_i]
            qsl = slice(qw * QW, (qw + 1) * QW)
            dens, raws = [], []
            # fast PSUM release: pull den (f32) + raw outT (bf16) out of the
            # pv banks first so the next window's PV accumulation can start.
            for h2 in range(2):
                pv = pv_ps.pop((w_i, h2))
                den = small.tile([1, QW], F32, name=f"den_{w_i}_{h2}", tag="den", bufs=3)
                nc.vector.tensor_copy(den[:], pv[DK:DK + 1, :])
                raw = small.tile([DK, QW], BF, name=f"raw_{w_i}_{h2}", tag="raw", bufs=3)
                nc.vector.tensor_copy(raw[:], pv[0:DK, :])
                dens.append(den)
                raws.append(raw)
            for h2 in range(2):
                rec = small.tile([1, QW], F32, name=f"rec_{w_i}_{h2}", tag="rec", bufs=3)
                nc.vector.reciprocal_approx_fast(rec[:], dens[h2][:])
                bc = small.tile([DK, QW], F32, name=f"bc_{w_i}_{h2}", tag="bc", bufs=2)
                nc.gpsimd.partition_broadcast(bc[:], rec[:])
                nc.vector.tensor_mul(
                    outT[hp][h2 * DK:(h2 + 1) * DK, qsl], raws[h2][:], bc[:]
                )

        NSTEP = NW * SM
        fill = iter(())
        emit_scores(0)
        for j in range(1, NSTEP):
            emit_scores(j)
            emit_pv(j - 1)
            w_prev, kt_prev = divmod(j - 1, SM)
            if kt_prev == SM - 1:
                emit_norm(w_prev)
                hp_p, qw_p = wins[w_prev]
                if hp_p == MF - 1:  # quarter qw_p fully done -> queue its W4
                    fill = gen_w4(range(qw_p * 4, (qw_p + 1) * 4))
            # consume filler only mid-window: its first MMs wait on the
            # previous window's normalize, which would head-of-line block
            # the PE queue at a window start.
            if j % SM >= 4:
                next(fill, None)
        emit_pv(NSTEP - 1)
        emit_norm(NW - 1)
        for _ in fill:
            pass
        for _ in gen_w4(range((NQW - 1) * 4, NQW * 4), tail=True):
            pass


_NC_CACHE = None


def _get_nc():
    global _NC_CACHE
    if _NC_CACHE is None:
        _NC_CACHE = _build_kernel()
    return _NC_CACHE


def _make_in_maps(query, key, value, W1, b1, W2, b2, W3, b3, W4, b4):
    in_maps = []
    ident = np.eye(P, dtype=BF16)
    for c in range(N_CORES):
        b, g = divmod(c, 4)
        gs = slice(g * F, (g + 1) * F)
        in_maps.append({
            "xq_t": np.ascontiguousarray(query[b].T).astype(BF16),
            "xk_t": np.ascontiguousarray(key[b].T).astype(BF16),
            "xv_t": np.ascontiguousarray(value[b].T).astype(BF16),
            "w1t": np.ascontiguousarray(W1[gs, :].T).astype(BF16),
            "w2t": np.ascontiguousarray(W2[gs, :].T).astype(BF16),
            "w3t": np.ascontiguousarray(W3[gs, :].T).astype(BF16),
            "w4t": np.ascontiguousarray(W4[:, gs].T).astype(BF16),
            "b1c": np.ascontiguousarray(b1[gs].reshape(F // P, P).T).astype(np.float32),
            "b2c": np.ascontiguousarray(b2[gs].reshape(F // P, P).T).astype(np.float32),
            "b3c": np.ascontiguousarray(b3[gs].reshape(F // P, P).T).astype(np.float32),
            "ident": ident,
        })
    return in_maps


def kernel(query, key, value, W1, b1, W2, b2, W3, b3, W4, b4, _trace=False, _tmpdir=None):
    args = [np.asarray(a) for a in (query, key, value, W1, b1, W2, b2, W3, b3, W4, b4)]
    nc = _get_nc()
    in_maps = _make_in_maps(*args)
    res = run_bass_kernel_spmd(
        nc, in_maps, core_ids=list(range(N_CORES)),
        trace=_trace, tmpdir=_tmpdir,
    )
    b4_f = args[10].astype(np.float32)
    full = np.zeros((B, S, D), np.float32)
    for c in range(N_CORES):
        full[c // 4] += res.results[c]["out"].astype(np.float32)
    full += b4_f[None, None, :]
    kernel.last_results = res
    return full


# revision 15
# speedup vs baseline: 1.6599x; 1.0111x over previous
"""Multi-head attention (B=2, S=2048, D=1024, H=16, d_k=64) on 8 NeuronCores.

Sharding: 8 cores = 2 batches x 4 head-groups (4 heads each).
Core c handles batch b = c//4 and heads 4*(c%4) .. 4*(c%4)+4 (feature
slice of width F=256). Each core computes its partial output-projection
contribution [S, D] in bf16; the host sums the 4 head-group partials per
batch in f32 and adds b4.

Device dataflow ("transposed world", zero-layout-change matmuls):
  qT = W1g @ x_q.T  [F, S]      kT = W2g @ x_k.T  [F, S]
  vT = W3g @ x_v.T  [F, S]  -> PE-transposed per 128-block into
  v   [S, F] with interleaved ones columns (softmax denominator trick)
  scoresT = kT_h.T @ qT_h  [S_keys, 512q x 2 heads packed]  (K=64; the two
            head MMs are a row-tiled concurrent pair, tile_position (0,0)/(64,0))
  attnT = exp(scoresT / 8)  one ACT instr per key tile, FD=1024
  pv = [v_h | ones].T @ attnT  [65, 512]; row 64 = denominator
  outT_h = pv[0:64] * recip(pv[64])  (reciprocal_approx_fast + gpsimd bcast)
  partial = outT.T @ W4g.T  [S, D] interleaved into later windows as PE filler

Schedule: the 8 attention windows (2 head-pairs x 4 query-quarters) run as
ONE flat software-pipelined stream over 128 (window, key-tile) steps:
scores(j+1) is emitted before pv(j), so the PE keeps streaming across
window boundaries and the ScalarE exp pipe (the ~140us roofline engine)
never gaps. DMA order xk | xv | xq so each projection starts as its
inputs land. PSUM: sc 2x2 banks + pv 2x1 + w4 2x1 = 8 banks exactly.
"""

import numpy as np
import ml_dtypes

import concourse.bass as bass
import concourse.mybir as mybir
import concourse.tile as tile
from concourse import bacc
from concourse.bass_utils import run_bass_kernel_spmd

BF16 = ml_dtypes.bfloat16
F32 = mybir.dt.float32
BF = mybir.dt.bfloat16

B, S, D = 2, 2048, 1024
H_CORE = 4          # heads per core
DK = 64             # head dim
F = H_CORE * DK     # features per core = 256
P = 128             # partitions
KB = D // P         # k blocks in D contraction = 8
SM = S // P         # seq tiles of 128 = 16
QW = 512            # query window width (per head)
NQW = S // QW       # query quarters = 4
N_CORES = 8
VW = H_CORE * (DK + 1)  # v with interleaved ones columns = 260


def _build_kernel():
    nc = bacc.Bacc(
        "TRN2",
        target_bir_lowering=False,
        debug=False,
        enable_asserts=False,
        num_devices=N_CORES,
    )

    xq = nc.dram_tensor("xq_t", [D, S], BF, kind="ExternalInput").ap()
    xk = nc.dram_tensor("xk_t", [D, S], BF, kind="ExternalInput").ap()
    xv = nc.dram_tensor("xv_t", [D, S], BF, kind="ExternalInput").ap()
    w1 = nc.dram_tensor("w1t", [D, F], BF, kind="ExternalInput").ap()
    w2 = nc.dram_tensor("w2t", [D, F], BF, kind="ExternalInput").ap()
    w3 = nc.dram_tensor("w3t", [D, F], BF, kind="ExternalInput").ap()
    w4 = nc.dram_tensor("w4t", [F, D], BF, kind="ExternalInput").ap()
    b1 = nc.dram_tensor("b1c", [P, F // P], F32, kind="ExternalInput").ap()
    b2 = nc.dram_tensor("b2c", [P, F // P], F32, kind="ExternalInput").ap()
    b3 = nc.dram_tensor("b3c", [P, F // P], F32, kind="ExternalInput").ap()
    ident = nc.dram_tensor("ident", [P, P], BF, kind="ExternalInput").ap()
    out = nc.dram_tensor("out", [S, D], BF, kind="ExternalOutput").ap()

    with tile.TileContext(nc) as tc:
        _body(tc, xq, xk, xv, w1, w2, w3, w4, b1, b2, b3, ident, out)

    nc.compile()
    return nc


def _body(tc, xq, xk, xv, w1, w2, w3, w4, b1, b2, b3, ident, out):
    nc = tc.nc
    MF = F // P  # head-pair tiles in the F=256 feature dim = 2
    EXP = mybir.ActivationFunctionType.Exp

    with (
        tc.tile_pool(name="wpool", bufs=1) as wpool,
        tc.tile_pool(name="xt", bufs=1) as xt_pool,
        tc.tile_pool(name="persist", bufs=1) as persist,
        tc.tile_pool(name="attn", bufs=6) as attn_pool,
        tc.tile_pool(name="small", bufs=4) as small,
        tc.tile_pool(name="stage", bufs=2) as stage,
        tc.tile_pool(name="psum", bufs=1, space="PSUM") as psum,
    ):
        # ---- weight / constant holder tiles ----
        w1_sb = [wpool.tile([P, F], BF, name=f"w1_{k}", tag=f"w1_{k}") for k in range(KB)]
        w2_sb = [wpool.tile([P, F], BF, name=f"w2_{k}", tag=f"w2_{k}") for k in range(KB)]
        w3_sb = [wpool.tile([P, F], BF, name=f"w3_{k}", tag=f"w3_{k}") for k in range(KB)]
        w4_sb = [wpool.tile([P, D], BF, name=f"w4_{k}", tag=f"w4_{k}") for k in range(MF)]
        b1_sb = wpool.tile([P, MF], F32, name="b1_sb", tag="b1_sb")
        b2_sb = wpool.tile([P, MF], F32, name="b2_sb", tag="b2_sb")
        b3_sb = wpool.tile([P, MF], F32, name="b3_sb", tag="b3_sb")
        id_sb = wpool.tile([P, P], BF, name="id_sb", tag="id_sb")

        # persistent activations
        qT = [persist.tile([P, S], BF, name=f"qT_{m}", tag=f"qT_{m}") for m in range(MF)]
        kT = [persist.tile([P, S], BF, name=f"kT_{m}", tag=f"kT_{m}") for m in range(MF)]
        vT = [persist.tile([P, S], BF, name=f"vT_{m}", tag=f"vT_{m}") for m in range(MF)]
        v_sb = [persist.tile([P, VW], BF, name=f"v_{s}", tag=f"v_{s}") for s in range(SM)]
        for s in range(SM):
            for h in range(H_CORE):
                nc.vector.memset(v_sb[s][:, h * (DK + 1) + DK: h * (DK + 1) + DK + 1], 1.0)
        outT = [persist.tile([P, S], BF, name=f"outT_{m}", tag=f"outT_{m}") for m in range(MF)]

        # ---- DMA order: k inputs, then v, then q, so each projection
        #      starts as soon as its tiles land ----
        def dma_w(w_sb_list, w_dram, nk):
            for k in range(nk):
                nc.sync.dma_start(w_sb_list[k][:], w_dram[k * P:(k + 1) * P, :])

        def dma_x(name, x_dram):
            ts = []
            for k in range(KB):
                t = xt_pool.tile([P, S], BF, name=f"x{name}_{k}", tag=f"xt_{name}_{k}",
                                 bufs=1)
                nc.sync.dma_start(t[:], x_dram[k * P:(k + 1) * P, :])
                ts.append(t)
            return ts

        dma_w(w2_sb, w2, KB)
        nc.sync.dma_start(b2_sb[:], b2[:])
        xk_sb = dma_x("k", xk)
        dma_w(w3_sb, w3, KB)
        nc.sync.dma_start(b3_sb[:], b3[:])
        nc.sync.dma_start(id_sb[:], ident[:])
        xv_sb = dma_x("v", xv)
        dma_w(w1_sb, w1, KB)
        nc.sync.dma_start(b1_sb[:], b1[:])
        xq_sb = dma_x("q", xq)
        dma_w(w4_sb, w4, MF)

        # ---- projections to transposed layout [F-slice, S] ----
        # The 4 query-quarter MMs per (m, k) share the same stationary weights;
        # skip the redundant LDWEIGHTS on the last 3. Bias rides on ScalarE
        # (idle during projections) instead of the DVE.
        IDENT = mybir.ActivationFunctionType.Identity

        def proj_qk(x_sb, w_sb, b_sb, dst):
            for m in range(MF):
                ps = [psum.tile([P, 1024], F32, name=f"pp_{m}_{i}", tag="sc", bufs=2)
                      for i in range(2)]
                for k in range(KB):
                    for qq in range(4):
                        mm = nc.tensor.matmul(
                            ps[qq // 2][:, (qq % 2) * 512:(qq % 2 + 1) * 512],
                            w_sb[k][:, m * P:(m + 1) * P],
                            x_sb[k][:, qq * 512:(qq + 1) * 512],
                            start=(k == 0),
                            stop=(k == KB - 1),
                        )
                        if qq > 0:
                            mm.ins.ldweights = False
                for i in range(2):
                    nc.scalar.activation(
                        dst[m][:, i * 1024:(i + 1) * 1024], ps[i][:], IDENT,
                        bias=b_sb[:, m:m + 1],
                    )

        proj_qk(xk_sb, w2_sb, b2_sb, kT)
        proj_qk(xv_sb, w3_sb, b3_sb, vT)

        # vT -> v: PE transpose each [128,128] block, then cast the two
        # 64-wide head slices into v_sb around the ones columns.
        for s in range(SM):
            for m in range(MF):
                tp = psum.tile([P, P], BF, name=f"tp_{s}_{m}", tag="w4", bufs=2)
                nc.tensor.transpose(tp[:], vT[m][:, s * P:(s + 1) * P], id_sb[:])
                for hh in range(2):
                    h = m * 2 + hh
                    nc.vector.tensor_copy(
                        v_sb[s][:, h * (DK + 1): h * (DK + 1) + DK],
                        tp[:, hh * DK:(hh + 1) * DK],
                    )

        proj_qk(xq_sb, w1_sb, b1_sb, qT)

        # ---- output projection generator (PE filler inside windows).
        #      tail=True routes the PSUM->SBUF copies to ScalarE (idle once
        #      the exps are done) to shorten the final-quarter tail. ----
        def gen_w4(qts, tail=False):
            for qt in qts:
                ob = stage.tile([P, D], BF, name=f"ob_{qt}", tag="ob", bufs=2)
                for oc in range(2):
                    ps = psum.tile([P, 512], F32, name=f"po_{qt}_{oc}", tag="w4", bufs=2)
                    for m in range(MF):
                        nc.tensor.matmul(
                            ps[:],
                            outT[m][:, qt * P:(qt + 1) * P],
                            w4_sb[m][:, oc * 512:(oc + 1) * 512],
                            start=(m == 0),
                            stop=(m == MF - 1),
                        )
                    if tail:
                        nc.scalar.copy(ob[:, oc * 512:(oc + 1) * 512], ps[:])
                    else:
                        nc.vector.tensor_copy(ob[:, oc * 512:(oc + 1) * 512], ps[:])
                    yield
                nc.sync.dma_start(out[qt * P:(qt + 1) * P, :], ob[:])
                yield

        # ---- attention: one flat pipelined stream over all 8 windows ----
        wins = [(hp, qw) for qw in range(NQW) for hp in range(MF)]
        NW = len(wins)
        pv_ps = {}      # (w_i, h2) -> psum tile
        attn_t = {}     # j -> attn tile

        def emit_scores(j):
            w_i, kt = divmod(j, SM)
            hp, qw = wins[w_i]
            sc = psum.tile([P, 1024], F32, name=f"sc_{j}", tag="sc", bufs=2)
            for h2 in range(2):
                rsl = slice(h2 * DK, (h2 + 1) * DK)
                nc.tensor.matmul(
                    sc[:, h2 * 512:(h2 + 1) * 512],
                    kT[hp][rsl, kt * P:(kt + 1) * P],
                    qT[hp][rsl, qw * QW:(qw + 1) * QW],
                    start=True,
                    stop=True,
                )
            at = attn_pool.tile([P, 1024], BF, name=f"at_{j}", tag="attnT", bufs=6)
            nc.scalar.activation(at[:], sc[:], EXP, scale=1.0 / np.sqrt(DK))
            attn_t[j] = at

        def emit_pv(j):
            w_i, kt = divmod(j, SM)
            hp, qw = wins[w_i]
            if kt == 0:
                for h2 in range(2):
                    pv_ps[(w_i, h2)] = psum.tile(
                        [DK + 1, QW], F32, name=f"pvps_{w_i}_{h2}", tag="pv", bufs=2)
            for h2 in range(2):
                h = hp * 2 + h2
                vsl = slice(h * (DK + 1), h * (DK + 1) + DK + 1)
                nc.tensor.matmul(
                    pv_ps[(w_i, h2)][:],
                    v_sb[kt][:, vsl],
                    attn_t[j][:, h2 * 512:(h2 + 1) * 512],
                    start=(kt == 0),
                    stop=(kt == SM - 1),
                )
            del attn_t[j]

        def emit_norm(w_i, tail=False):
            hp, qw = wins[w_i]
            qsl = slice(qw * QW, (qw + 1) * QW)
            # fast PSUM release + earliest possible outT readiness: per head,
            # den copy -> recip -> bf16 cast (2x-mode mul) -> bcast -> mul.
            # tail=True moves the PSUM pulls to ScalarE (idle after last exp).
            for h2 in range(2):
                pv = pv_ps.pop((w_i, h2))
                den = small.tile([1, QW], F32, name=f"den_{w_i}_{h2}", tag="den", bufs=3)
                raw = small.tile([DK, QW], BF, name=f"raw_{w_i}_{h2}", tag="raw", bufs=3)
                if tail:
                    nc.scalar.copy(den[:], pv[DK:DK + 1, :])
                    nc.scalar.copy(raw[:], pv[0:DK, :])
                else:
                    nc.vector.tensor_copy(den[:], pv[DK:DK + 1, :])
                    nc.vector.tensor_copy(raw[:], pv[0:DK, :])
                rec = small.tile([1, QW], F32, name=f"rec_{w_i}_{h2}", tag="rec", bufs=3)
                nc.vector.reciprocal_approx_fast(rec[:], den[:])
                rb = small.tile([1, QW], BF, name=f"rb_{w_i}_{h2}", tag="rb", bufs=3)
                nc.vector.tensor_copy(rb[:], rec[:])
                bc = small.tile([DK, QW], BF, name=f"bc_{w_i}_{h2}", tag="bc", bufs=2)
                nc.gpsimd.partition_broadcast(bc[:], rb[:])
                nc.vector.tensor_mul(
                    outT[hp][h2 * DK:(h2 + 1) * DK, qsl], raw[:], bc[:]
                )

        NSTEP = NW * SM
        fill = iter(())
        emit_scores(0)
        for j in range(1, NSTEP):
            emit_scores(j)
            emit_pv(j - 1)
            w_prev, kt_prev = divmod(j - 1, SM)
            if kt_prev == SM - 1:
                emit_norm(w_prev)
                hp_p, qw_p = wins[w_prev]
                if hp_p == MF - 1:  # quarter qw_p fully done -> queue its W4
                    fill = gen_w4(range(qw_p * 4, (qw_p + 1) * 4))
            # consume filler only late in the window: its first MMs wait on
            # the previous window's normalize muls, which would head-of-line
            # block the PE queue earlier.
            if j % SM >= 7:
                next(fill, None)
        emit_pv(NSTEP - 1)
        emit_norm(NW - 1, tail=True)
        for _ in fill:
            pass
        for _ in gen_w4(range((NQW - 1) * 4, NQW * 4), tail=True):
            pass


_NC_CACHE = None


def _get_nc():
    global _NC_CACHE
    if _NC_CACHE is None:
        _NC_CACHE = _build_kernel()
    return _NC_CACHE


def _make_in_maps(query, key, value, W1, b1, W2, b2, W3, b3, W4, b4):
    in_maps = []
    ident = np.eye(P, dtype=BF16)
    for c in range(N_CORES):
        b, g = divmod(c, 4)
        gs = slice(g * F, (g + 1) * F)
        in_maps.append({
            "xq_t": np.ascontiguousarray(query[b].T).astype(BF16),
            "xk_t": np.ascontiguousarray(key[b].T).astype(BF16),
            "xv_t": np.ascontiguousarray(value[b].T).astype(BF16),
            "w1t": np.ascontiguousarray(W1[gs, :].T).astype(BF16),
            "w2t": np.ascontiguousarray(W2[gs, :].T).astype(BF16),
            "w3t": np.ascontiguousarray(W3[gs, :].T).astype(BF16),
            "w4t": np.ascontiguousarray(W4[:, gs].T).astype(BF16),
            "b1c": np.ascontiguousarray(b1[gs].reshape(F // P, P).T).astype(np.float32),
            "b2c": np.ascontiguousarray(b2[gs].reshape(F // P, P).T).astype(np.float32),
            "b3c": np.ascontiguousarray(b3[gs].reshape(F // P, P).T).astype(np.float32),
            "ident": ident,
        })
    return in_maps


def kernel(query, key, value, W1, b1, W2, b2, W3, b3, W4, b4, _trace=False, _tmpdir=None):
    args = [np.asarray(a) for a in (query, key, value, W1, b1, W2, b2, W3, b3, W4, b4)]
    nc = _get_nc()
    in_maps = _make_in_maps(*args)
    res = run_bass_kernel_spmd(
        nc, in_maps, core_ids=list(range(N_CORES)),
        trace=_trace, tmpdir=_tmpdir,
    )
    b4_f = args[10].astype(np.float32)
    full = np.zeros((B, S, D), np.float32)
    for c in range(N_CORES):
        full[c // 4] += res.results[c]["out"].astype(np.float32)
    full += b4_f[None, None, :]
    kernel.last_results = res
    return full
